# revision 1
# baseline (speedup 1.0000x reference)
"""BigBird sparse attention on 8 Trainium2 NeuronCores.

Sharding: batch*heads = 64 (b,h) pairs, 8 per core (data parallel, no
collectives). On-core, pairs are processed two at a time ("A"/"B")
stacked on SBUF partitions 0-63 / 64-127: with tile_position quadrant
packing the PE runs A's and B's matmuls concurrently in disjoint
regions of the 128x128 array, and ScalarE exp() always sees full-height
[128, x] tiles.

Host prep: Q/K transposed to [D, T]; random K/V gathered by rand_idx;
V augmented with a ones column (so the softmax denominator falls out of
the AV matmul as row D); V variants stacked per pair-duo.

Per pair-duo:
  Stage A (global queries 0..63): S_g^T chunks [128k, 64q] per pair
    (B's matmuls at tile_position=(64,0), row-concurrent with A's),
    exp, then K=128 accumulated AV matmuls -> out_g^T [65, 64],
    PE-transpose back, normalize, DMA.
  Stage B (63 local blocks in chunks of <=8 blocks / 512 queries):
    scores keys-on-partitions in 5 full-height PSUM tiles
    {local, global, rnd0, rnd1, rnd2} with A in partitions 0-63 and B
    in 64-127 (B at tile_position=(64,64)); one exp per tile; AV into
    per-pair out^T [65, cq] accumulators (B at tile_position=(64,0));
    PE-transpose to [q, 65]; normalize by reciprocal of column 64; DMA.

Softmax skips max-subtraction: scores/sqrt(D) are ~N(0,1) for randn
inputs, so exp stays comfortably inside fp32 range and normalization
cancels the shift exactly in exact math.

All matmul inputs are bf16 (fp32 PSUM accumulation); everything after
exp stays fp32 through normalization.
"""

import numpy as np

B, T, H, D = 4, 4096, 16, 64
BS, G, R = 64, 64, 192
NCORE = 8
BH = B * H
NPAIR = BH // NCORE          # 8 pairs per core
NSP = NPAIR // 2             # 4 stacked pair-duos per core
NB = (T - G) // BS           # 63 local blocks
INV_SCALE = float(D) ** -0.5
DA = D + 1                   # V augmented with ones column
NKC = T // 128               # 32 key chunks of 128
NR = R // BS                 # 3 random-key groups of 64

_PROGRAM_CACHE = {}


def _body(ctx, tc, din, out):
    import concourse.mybir as mybir
    from concourse.masks import make_identity

    nc = tc.nc
    f32 = mybir.dt.float32
    bf16 = mybir.dt.bfloat16
    EXP = mybir.ActivationFunctionType.Exp

    consts = ctx.enter_context(tc.tile_pool(name="consts", bufs=1))
    pin = ctx.enter_context(tc.tile_pool(name="pin", bufs=2))
    pe = ctx.enter_context(tc.tile_pool(name="pe", bufs=4))
    pegt = ctx.enter_context(tc.tile_pool(name="pegt", bufs=3))
    psS = ctx.enter_context(tc.tile_pool(name="psS", bufs=1, space="PSUM"))
    psO = ctx.enter_context(tc.tile_pool(name="psO", bufs=1, space="PSUM"))
    psT = ctx.enter_context(tc.tile_pool(name="psT", bufs=1, space="PSUM"))

    ident = consts.tile([128, 128], f32)
    make_identity(nc, ident)

    # block chunking: 7 chunks of 8 blocks + 1 chunk of 7 blocks
    chunks = []
    n0 = 0
    while n0 < NB:
        nb = min(8, NB - n0)
        chunks.append((n0, nb))
        n0 += nb

    halves = ((0, slice(0, 64)), (1, slice(64, 128)))

    for sp in range(NSP):
        pA, pB = 2 * sp, 2 * sp + 1
        # ---- load stacked inputs ----
        qt2 = pin.tile([128, T], bf16, tag="qt2")
        kt2 = pin.tile([128, T], bf16, tag="kt2")
        krt2 = pin.tile([128, R], bf16, tag="krt2")
        vg2 = pin.tile([128, DA], bf16, tag="vg2")
        vr2 = pin.tile([128, NR, DA], bf16, tag="vr2")
        vb2 = pin.tile([128, NB, DA], bf16, tag="vb2")
        vcA = pin.tile([128, NKC, DA], bf16, tag="vcA")
        vcB = pin.tile([128, NKC, DA], bf16, tag="vcB")
        nc.sync.dma_start(out=qt2[0:64, :], in_=din["qT"][pA])
        nc.sync.dma_start(out=qt2[64:128, :], in_=din["qT"][pB])
        nc.sync.dma_start(out=kt2[0:64, :], in_=din["kT"][pA])
        nc.sync.dma_start(out=kt2[64:128, :], in_=din["kT"][pB])
        nc.sync.dma_start(out=krt2[0:64, :], in_=din["krT"][pA])
        nc.sync.dma_start(out=krt2[64:128, :], in_=din["krT"][pB])
        nc.sync.dma_start(out=vg2, in_=din["vgs"][sp])
        nc.sync.dma_start(out=vr2, in_=din["vrs"][sp])
        nc.sync.dma_start(out=vb2, in_=din["vbs"][sp])
        nc.sync.dma_start(out=vcA, in_=din["vch"][pA])
        nc.sync.dma_start(out=vcB, in_=din["vch"][pB])

        # ---- Stage A: global queries ----
        egtA = pegt.tile([128, NKC, G], bf16, tag="egtA")
        egtB = pegt.tile([128, NKC, G], bf16, tag="egtB")
        sgt_tags = (("s_loc", "s_glo"), ("s_r0", "s_r1"))
        KCG = 8  # key chunks per psum tile; one exp per group
        for g in range(NKC // KCG):
            tagA, tagB = sgt_tags[g % 2]
            sgtA = psS.tile([128, KCG, G], f32, tag=tagA, name="sgtA")
            sgtB = psS.tile([128, KCG, G], f32, tag=tagB, name="sgtB")
            for i in range(KCG):
                ko = 128 * (g * KCG + i)
                nc.tensor.matmul(
                    sgtA[:, i, :],
                    kt2[0:64, ko : ko + 128],
                    qt2[0:64, 0:G],
                    start=(i == 0),
                    stop=(i == KCG - 1),
                )
                nc.tensor.matmul(
                    sgtB[:, i, :],
                    kt2[64:128, ko : ko + 128],
                    qt2[64:128, 0:G],
                    tile_position=(64, 0),
                    start=(i == 0),
                    stop=(i == KCG - 1),
                )
            nc.scalar.activation(
                egtA[:, g * KCG : (g + 1) * KCG, :], sgtA, EXP, scale=INV_SCALE
            )
            nc.scalar.activation(
                egtB[:, g * KCG : (g + 1) * KCG, :], sgtB, EXP, scale=INV_SCALE
            )
        outgA = psO.tile([DA, G], f32, tag="pout_A")
        outgB = psO.tile([DA, G], f32, tag="pout_B")
        for kc in range(NKC):
            nc.tensor.matmul(
                outgA,
                vcA[:, kc, :],
                egtA[:, kc, :],
                start=(kc == 0),
                stop=(kc == NKC - 1),
            )
            nc.tensor.matmul(
                outgB,
                vcB[:, kc, :],
                egtB[:, kc, :],
                start=(kc == 0),
                stop=(kc == NKC - 1),
            )
        for p, outg in ((pA, outgA), (pB, outgB)):
            outg_sb = pe.tile([DA, G], f32, tag="poutsA")
            nc.vector.tensor_copy(outg_sb, outg)
            outg2 = psT.tile([G, DA], f32, tag="pt")
            nc.tensor.transpose(outg2, outg_sb, ident[:DA, :DA])
            recg = pe.tile([G, 1], f32, tag="recipA")
            nc.vector.reciprocal(recg, outg2[:, D : D + 1])
            outg_n = pe.tile([G, D], f32, tag="outnA")
            nc.vector.tensor_scalar_mul(outg_n, outg2[:, 0:D], recg)
            nc.sync.dma_start(out=out[p, 0:G, :], in_=outg_n)

        # ---- Stage B: block queries ----
        for n0, nb in chunks:
            cq = BS * nb
            qoff = G + BS * n0

            ploc = psS.tile([128, 512], f32, tag="s_loc")
            pglo = psS.tile([128, 512], f32, tag="s_glo")
            prnd = [
                psS.tile([128, 512], f32, tag=f"s_r{j}", name=f"prnd{j}")
                for j in range(NR)
            ]

            # Two waves so same-bank A/B groups never interleave, while
            # adjacent instructions still hit disjoint array quadrants:
            # wave 1: A-local stream ||| B-global/random; wave 2 swapped.
            def s_loc_mm(hi, rows, j):
                koff = G + BS * (n0 + j)
                nc.tensor.matmul(
                    ploc[rows, BS * j : BS * (j + 1)],
                    kt2[rows, koff : koff + BS],
                    qt2[rows, koff : koff + BS],
                    tile_position=(64, 64) if hi else None,
                    start=(j == 0),
                    stop=(j == nb - 1),
                )

            def s_big_mms(hi, rows):
                tp = (64, 64) if hi else None
                yield lambda: nc.tensor.matmul(
                    pglo[rows, 0:cq],
                    kt2[rows, 0:G],
                    qt2[rows, qoff : qoff + cq],
                    tile_position=tp,
                    start=True,
                    stop=True,
                )
                for j in range(NR):
                    yield (
                        lambda j=j: nc.tensor.matmul(
                            prnd[j][rows, 0:cq],
                            krt2[rows, BS * j : BS * (j + 1)],
                            qt2[rows, qoff : qoff + cq],
                            tile_position=tp,
                            start=True,
                            stop=True,
                        )
                    )

            for wave in range(2):
                hi_loc, rows_loc = halves[wave]
                hi_big, rows_big = halves[1 - wave]
                big = list(s_big_mms(hi_big, rows_big))
                for j in range(nb):
                    s_loc_mm(hi_loc, rows_loc, j)
                    if j < len(big):
                        big[j]()

            eloc = pe.tile([128, 512], bf16, tag="eloc")
            eglo = pe.tile([128, 512], bf16, tag="eglo")
            ernd = [
                pe.tile([128, 512], bf16, tag=f"er{j}", name=f"ernd{j}")
                for j in range(NR)
            ]
            nc.scalar.activation(eglo[:, 0:cq], pglo[:, 0:cq], EXP, scale=INV_SCALE)
            for j in range(NR):
                nc.scalar.activation(
                    ernd[j][:, 0:cq], prnd[j][:, 0:cq], EXP, scale=INV_SCALE
                )
            nc.scalar.activation(eloc[:, 0:cq], ploc[:, 0:cq], EXP, scale=INV_SCALE)

            poutA = psO.tile([DA, 512], f32, tag="pout_A")
            poutB = psO.tile([DA, 512], f32, tag="pout_B")
            pouts = (poutA, poutB)
            for hi, rows in halves:
                nc.tensor.matmul(
                    pouts[hi][:, 0:cq],
                    vg2[rows, :],
                    eglo[rows, 0:cq],
                    tile_position=(64, 0) if hi else None,
                    start=True,
                    stop=False,
                )
            for j in range(NR):
                for hi, rows in halves:
                    nc.tensor.matmul(
                        pouts[hi][:, 0:cq],
                        vr2[rows, j, :],
                        ernd[j][rows, 0:cq],
                        tile_position=(64, 0) if hi else None,
                        start=False,
                        stop=False,
                    )
            for j in range(nb):
                n = n0 + j
                for hi, rows in halves:
                    nc.tensor.matmul(
                        pouts[hi][:, BS * j : BS * (j + 1)],
                        vb2[rows, n, :],
                        eloc[rows, BS * j : BS * (j + 1)],
                        tile_position=(64, 0) if hi else None,
                        start=False,
                        stop=(j == nb - 1),
                    )

            nt = (cq + 127) // 128
            pout_sbs = []
            for hi in (0, 1):
                pout_sb = pe.tile([DA, 512], f32, tag=f"pouts{hi}", name="pout_sb")
                nc.vector.tensor_copy(pout_sb[:, 0:cq], pouts[hi][:, 0:cq])
                pout_sbs.append(pout_sb)
            for hi, p in ((0, pA), (1, pB)):
                pout_sb = pout_sbs[hi]
                pt = psT.tile([128, nt, DA], f32, tag="pt")
                for t in range(nt):
                    w = min(128, cq - 128 * t)
                    nc.tensor.matmul(
                        pt[0:w, t, 0:DA],
                        pout_sb[:, 128 * t : 128 * t + w],
                        ident[:DA, :DA],
                        is_transpose=True,
                        start=True,
                        stop=True,
                    )
                outn = pe.tile([128, nt, D], f32, tag="outn")
                for t in range(nt):
                    w = min(128, cq - 128 * t)
                    rec = pe.tile([128, 1], f32, tag="recip")
                    nc.vector.reciprocal(
                        rec[0:w], pt[0:w, t, D : D + 1]
                    )
                    nc.vector.tensor_scalar_mul(
                        outn[0:w, t, :], pt[0:w, t, 0:D], rec[0:w]
                    )
                nfull = cq // 128
                nc.sync.dma_start(
                    out=out[p, qoff : qoff + 128 * nfull, :].rearrange(
                        "(t pp) d -> pp t d", pp=128
                    ),
                    in_=outn[:, 0:nfull, :],
                )
                if cq > 128 * nfull:
                    rem = cq - 128 * nfull
                    nc.sync.dma_start(
                        out=out[p, qoff + 128 * nfull : qoff + cq, :],
                        in_=outn[0:rem, nfull, :],
                    )


def _build_program():
    from contextlib import ExitStack

    import concourse.bacc as bacc
    import concourse.mybir as mybir
    import concourse.tile as tile

    bf16 = mybir.dt.bfloat16
    nc = bacc.Bacc(
        "TRN2", target_bir_lowering=False, debug=False, num_devices=NCORE
    )
    shapes = {
        "qT": [NPAIR, D, T],
        "kT": [NPAIR, D, T],
        "krT": [NPAIR, D, R],
        "vch": [NPAIR, 128, NKC, DA],
        "vgs": [NSP, 128, DA],
        "vrs": [NSP, 128, NR, DA],
        "vbs": [NSP, 128, NB, DA],
    }
    din = {
        name: nc.dram_tensor(name, shp, bf16, kind="ExternalInput").ap()
        for name, shp in shapes.items()
    }
    out = nc.dram_tensor(
        "out", [NPAIR, T, D], mybir.dt.float32, kind="ExternalOutput"
    ).ap()

    with tile.TileContext(nc) as tc:
        with ExitStack() as ctx:
            _body(ctx, tc, din, out)
    nc.compile()
    return nc


def get_program():
    if "v3" not in _PROGRAM_CACHE:
        _PROGRAM_CACHE["v3"] = _build_program()
    return _PROGRAM_CACHE["v3"]


def prep_inputs(q, k, v, rand_idx):
    """Host-side shard + layout prep. Returns list of per-core input dicts."""
    import ml_dtypes

    bf16 = ml_dtypes.bfloat16
    idx = np.asarray(rand_idx).astype(np.int64)
    qp = np.ascontiguousarray(q.transpose(0, 2, 3, 1)).reshape(BH, D, T)
    kp = np.ascontiguousarray(k.transpose(0, 2, 3, 1)).reshape(BH, D, T)
    krTp = np.ascontiguousarray(kp[:, :, idx])  # [BH, D, R]
    vp = np.ascontiguousarray(v.transpose(0, 2, 1, 3)).reshape(BH, T, D)
    v_aug = np.concatenate([vp, np.ones((BH, T, 1), np.float32)], axis=2)
    vchp = np.ascontiguousarray(
        v_aug.reshape(BH, NKC, 128, DA).transpose(0, 2, 1, 3)
    )  # [BH, 128, NKC, DA]
    # stacked pair-duo variants: rows 0-63 = pair A, rows 64-127 = pair B
    vgs = v_aug[:, 0:G, :].reshape(BH // 2, 128, DA)
    vr = v_aug[:, idx, :].reshape(BH // 2, 2, NR, BS, DA)  # [sp, ab, j, key, da]
    vrs = np.ascontiguousarray(
        vr.transpose(0, 1, 3, 2, 4).reshape(BH // 2, 128, NR, DA)
    )
    vbs = np.ascontiguousarray(
        v_aug[:, G:, :].reshape(BH, NB, BS, DA).transpose(0, 2, 1, 3)
    ).reshape(BH // 2, 128, NB, DA)

    full = {
        "qT": qp,
        "kT": kp,
        "krT": krTp,
        "vch": vchp,
        "vgs": vgs,
        "vrs": vrs,
        "vbs": vbs,
    }
    in_maps = []
    for c in range(NCORE):
        m = {}
        for name, arr in full.items():
            per = arr.shape[0] // NCORE
            m[name] = np.ascontiguousarray(arr[c * per : (c + 1) * per]).astype(
                bf16
            )
        in_maps.append(m)
    return in_maps


def assemble_output(results):
    """[8 cores] x {"out": [NPAIR, T, D]} -> [B, T, H, D]"""
    full = np.concatenate([r["out"] for r in results], axis=0)  # [BH, T, D]
    return np.ascontiguousarray(
        full.reshape(B, H, T, D).transpose(0, 2, 1, 3)
    )


def kernel(q, k, v, rand_idx, _trace=False):
    from concourse.bass_utils import run_bass_kernel_spmd

    nc = get_program()
    in_maps = prep_inputs(
        np.asarray(q, dtype=np.float32),
        np.asarray(k, dtype=np.float32),
        np.asarray(v, dtype=np.float32),
        rand_idx,
    )
    res = run_bass_kernel_spmd(nc, in_maps, list(range(NCORE)), trace=_trace)
    out = assemble_output(res.results)
    if _trace:
        return out, res
    return out

